# revision 9
# baseline (speedup 1.0000x reference)
"""Trainium2 Bass kernel for nn_BrewCnnLayer (2-layer CNN + relu-ratio).

Math (per image, fp32):
  h1b = conv7x7_valid(x.reshape(28,28), W1)   -> (10, 22, 22)
  h1, rat1 = relu(h1b), step(h1b)
  h2b = conv5x5_valid(h1, W2)                 -> (10, 18, 18)
  h2, rat2 = relu(h2b), step(h2b)
returns (h2, rat1, rat2) for the full batch of 2048.

Mapping: pure data parallel over batch across 8 NeuronCores (256 images each).
On-chip, convs are matmuls on the PE array:
  conv1: K = (kx, y_in) Toeplitz patch partitions (<=126), M = (y_out, o) <= 120,
         N = (x, batch) streamed. Input is host-expanded into patch tensors (XE).
  conv2: K = (y_in, c) = h1 rows on partitions, M = (y_out, o), 5 accumulating
         matmuls over kx with shifted x-slices of the h1 tile.
conv1 output rows are grouped exactly as conv2 consumes them (rows 0-11 / 8-19 /
16-21) so the relu epilogue writes land directly in conv2's rhs layout.
rat1/rat2 are computed as uint8 on-chip (step function) to cut DMA volume.
"""

import contextlib
import os
import threading

import numpy as np

_NCORES = 8
_BFULL = 2048
_BC = _BFULL // _NCORES  # 256 images per core

# conv1 groups: (first h1 row, input image rows consumed, h1 rows produced)
_GROUPS = [(0, 18, 12), (8, 18, 12), (16, 12, 6)]
# conv2 blocks: (first h2 row, h1 rows consumed, h2 rows produced)
_BLOCKS = [(0, 12, 8), (8, 12, 8), (16, 6, 2)]
# rat1 slices: (group idx, first row within group, n rows) -> covers h1 rows 0..21 once
_R1CUTS = [(0, 0, 12), (1, 4, 8), (2, 4, 2)]

_lock = threading.Lock()
_cache = {}


# ---------------------------------------------------------------- host prep

def _fp32r_round(a):
    """Round fp32 values to the PE's fp32r format (11-bit mantissa,
    round-to-nearest-even) — bit-identical to neuronxcc's fp32_to_fp32r."""
    u = np.ascontiguousarray(a, np.float32).view(np.uint32)
    low12 = u & np.uint32(0xFFF)
    rnd = (low12 > 0x800) | ((low12 == 0x800)
                             & (((u >> np.uint32(12)) & np.uint32(1)) == np.uint32(1)))
    out = (u & np.uint32(0xFFFFF000)) + np.where(rnd, np.uint32(0x1000),
                                                 np.uint32(0)).astype(np.uint32)
    return out.view(np.float32)


def _build_lhsT1(W1r, Hg, Rg):
    """[7*Hg, Rg*10]: lhsT[(kx,yi),(yr,o)] = W1[o, yi-yr, kx]."""
    L = np.zeros((7, Hg, Rg, 10), np.float32)
    yi = np.arange(Hg)[:, None]
    yr = np.arange(Rg)[None, :]
    ky = yi - yr  # [Hg, Rg]
    valid = (ky >= 0) & (ky < 7)
    for kx in range(7):
        # W1r[:, ky, kx] -> [10] per (yi, yr)
        src = W1r[:, np.clip(ky, 0, 6), kx]          # [10, Hg, Rg]
        L[kx] = np.where(valid[:, :, None], src.transpose(1, 2, 0), 0.0)
    return L.reshape(7 * Hg, Rg * 10)


def _build_lhsT2(W2r, Hb, Ob):
    """[Hb*10, 5, Ob*10]: lhsT[(yi,c), kx, (yr,o)] = W2[o, c, yi-yr, kx]."""
    L = np.zeros((Hb, 10, 5, Ob, 10), np.float32)
    for yi in range(Hb):
        for yr in range(Ob):
            ky = yi - yr
            if 0 <= ky < 5:
                # W2r[o, c, ky, kx] -> [c, kx, o]
                L[yi, :, :, yr, :] = W2r[:, :, ky, :].transpose(1, 2, 0)
    return L.reshape(Hb * 10, 5, Ob * 10)


def _host_inputs(x, W1, W2):
    """Build per-core in_maps for run_bass_kernel_spmd."""
    x = np.ascontiguousarray(x, np.float32)
    W1r = np.asarray(W1, np.float32).reshape(10, 7, 7)
    W2r = np.asarray(W2, np.float32).reshape(10, 10, 5, 5)

    w1ab = _fp32r_round(_build_lhsT1(W1r, 18, 12))   # [126, 120]
    w1c = _fp32r_round(_build_lhsT1(W1r, 12, 6))     # [84, 60]
    w2ab = _fp32r_round(_build_lhsT2(W2r, 12, 8))    # [120, 5, 80]
    w2c = _fp32r_round(_build_lhsT2(W2r, 6, 2))      # [60, 5, 20]

    x2 = _fp32r_round(x).reshape(_BFULL, 28, 28)
    in_maps = []
    for c in range(_NCORES):
        xc = x2[c * _BC:(c + 1) * _BC]          # [BC, 28, 28]
        m = {"w1ab": w1ab, "w1c": w1c, "w2ab": w2ab, "w2c": w2c}
        for gi, (g0, Hg, _Rg) in enumerate(_GROUPS):
            xe = np.empty((7, Hg, 22, _BC), np.float32)
            for kx in range(7):
                xe[kx] = xc[:, g0:g0 + Hg, kx:kx + 22].transpose(1, 2, 0)
            m[f"xe{gi}"] = xe.reshape(7 * Hg, 22, _BC)
        in_maps.append(m)
    return in_maps


def _assemble(results):
    """Per-core result dicts -> (h2, rat1, rat2) full arrays."""
    h2 = np.empty((_BFULL, 10, 18, 18), np.float32)
    rat2 = np.empty((_BFULL, 10, 18, 18), np.float32)
    rat1 = np.empty((_BFULL, 10, 22, 22), np.float32)
    for c, r in enumerate(results):
        bs = slice(c * _BC, (c + 1) * _BC)
        for bi, (b0, _Hb, Ob) in enumerate(_BLOCKS):
            h2s = r[f"h2_{bi}"]                  # [(yr,o), 18, BC]
            r2s = r[f"r2_{bi}"]
            h2[bs, :, b0:b0 + Ob, :] = h2s.reshape(Ob, 10, 18, _BC).transpose(3, 1, 0, 2)
            rat2[bs, :, b0:b0 + Ob, :] = r2s.reshape(Ob, 10, 18, _BC).transpose(3, 1, 0, 2)
        for gi, yr0, nr in _R1CUTS:
            g0 = _GROUPS[gi][0]
            sl = r[f"r1_{gi}"]                   # [nr*10, 22, BC]
            rat1[bs, :, g0 + yr0:g0 + yr0 + nr, :] = sl.reshape(nr, 10, 22, _BC).transpose(3, 1, 0, 2)
    return h2, rat1, rat2


# ---------------------------------------------------------------- bass program

def _build_program():
    import concourse.bass as bass
    import concourse.tile as tile
    import concourse.mybir as mybir
    from concourse import bacc

    f32 = mybir.dt.float32
    f32r = mybir.dt.float32r
    u8 = mybir.dt.uint8
    Relu = mybir.ActivationFunctionType.Relu
    is_gt = mybir.AluOpType.is_gt

    nc = bacc.Bacc("TRN2", target_bir_lowering=False, debug=False,
                   num_devices=_NCORES)

    # DRAM I/O (matmul operands are pre-rounded fp32r, supplied by the host)
    xe_dr = [nc.dram_tensor(f"xe{gi}", [7 * Hg, 22, _BC], f32r, kind="ExternalInput").ap()
             for gi, (_g0, Hg, _Rg) in enumerate(_GROUPS)]
    w1ab_dr = nc.dram_tensor("w1ab", [126, 120], f32r, kind="ExternalInput").ap()
    w1c_dr = nc.dram_tensor("w1c", [84, 60], f32r, kind="ExternalInput").ap()
    w2ab_dr = nc.dram_tensor("w2ab", [120, 5, 80], f32r, kind="ExternalInput").ap()
    w2c_dr = nc.dram_tensor("w2c", [60, 5, 20], f32r, kind="ExternalInput").ap()

    h2_dr = [nc.dram_tensor(f"h2_{bi}", [Ob * 10, 18, _BC], f32, kind="ExternalOutput").ap()
             for bi, (_b0, _Hb, Ob) in enumerate(_BLOCKS)]
    r2_dr = [nc.dram_tensor(f"r2_{bi}", [Ob * 10, 18, _BC], u8, kind="ExternalOutput").ap()
             for bi, (_b0, _Hb, Ob) in enumerate(_BLOCKS)]
    r1_dr = [nc.dram_tensor(f"r1_{gi}", [nr * 10, 22, _BC], u8, kind="ExternalOutput").ap()
             for gi, _yr0, nr in _R1CUTS]

    with tile.TileContext(nc) as tc:
        with (
            tc.tile_pool(name="w", bufs=1) as wpool,
            tc.tile_pool(name="xe", bufs=2) as xepool,
            tc.tile_pool(name="h1", bufs=2) as h1pool,
            tc.tile_pool(name="r1", bufs=2) as r1pool,
            tc.tile_pool(name="h2", bufs=2) as h2pool,
            tc.tile_pool(name="r2", bufs=2) as r2pool,
            tc.tile_pool(name="ps1", bufs=2, space="PSUM") as ps1pool,
            tc.tile_pool(name="ps2", bufs=2, space="PSUM") as ps2pool,
        ):
            w1ab_sb = wpool.tile([126, 120], f32r)
            nc.sync.dma_start(w1ab_sb[:], w1ab_dr[:])
            w1c_sb = wpool.tile([84, 60], f32r)
            nc.sync.dma_start(w1c_sb[:], w1c_dr[:])
            w2ab_sb = wpool.tile([120, 5, 80], f32r)
            nc.sync.dma_start(w2ab_sb[:], w2ab_dr[:])
            w2c_sb = wpool.tile([60, 5, 20], f32r)
            nc.sync.dma_start(w2c_sb[:], w2c_dr[:])

            h1_tiles = [None, None, None]

            def conv1_group(gi):
                g0, Hg, Rg = _GROUPS[gi]
                K, M = 7 * Hg, Rg * 10
                w_sb = w1ab_sb if gi < 2 else w1c_sb
                xe_sb = xepool.tile([K, 22, _BC], f32r, tag="xe", name=f"xe_sb{gi}")
                for c0, c1 in ((0, 6), (6, 12), (12, 17), (17, 22)):
                    nc.sync.dma_start(xe_sb[:, c0:c1, :], xe_dr[gi][:, c0:c1, :])
                # h1 is fp32r: the relu epilogue rounds on write, so conv2 can
                # consume the tile directly as a matmul operand.
                h1t = h1pool.tile([M, 22, _BC], f32r, tag="h1", name=f"h1t{gi}")
                for s0 in range(0, 22, 4):
                    w = min(4, 22 - s0)
                    ps = ps1pool.tile([M, 4, _BC], f32, tag="ps1", name=f"ps1_{gi}_{s0}")
                    for j in range(0, w, 2):
                        nc.tensor.matmul(
                            ps[:, j:j + 2, :],
                            w_sb[:],
                            xe_sb[:, s0 + j:s0 + j + 2, :],
                            start=True, stop=True,
                        )
                    nc.scalar.activation(h1t[:, s0:s0 + w, :], ps[:, 0:w, :], Relu)
                h1_tiles[gi] = h1t
                r1t = r1pool.tile([M, 22, _BC], u8, tag="r1", name=f"r1t{gi}")
                nc.gpsimd.tensor_scalar(r1t[:], h1t[:].bitcast(f32), 0.0, None, is_gt)
                _gi, yr0, nr = _R1CUTS[gi]
                nc.sync.dma_start(r1_dr[gi][:], r1t[yr0 * 10:(yr0 + nr) * 10, :, :])

            def conv2_block(bi):
                b0, Hb, Ob = _BLOCKS[bi]
                K, M = Hb * 10, Ob * 10
                w_sb = w2ab_sb if bi < 2 else w2c_sb
                h1t = h1_tiles[bi]
                h2t = h2pool.tile([M, 18, _BC], f32, tag="h2", name=f"h2t{bi}")
                chunks = [(i * 28, 28) for i in range(9)] + [(252, 4)]
                for s in range(0, 10, 2):
                    ps = ps2pool.tile([M, 2, 512], f32, tag="ps2", name=f"ps2_{bi}_{s}")
                    for j in (0, 1):
                        bco, bw = chunks[s + j]
                        for kx in range(5):
                            nc.tensor.matmul(
                                ps[:, j, 0:18 * bw],
                                w_sb[:, kx, :],
                                h1t[:, kx:kx + 18, bco:bco + bw],
                                start=(kx == 0), stop=(kx == 4),
                            )
                        # relu epilogue (DVE) psum -> h2 sbuf
                        nc.vector.tensor_scalar_max(
                            h2t[:, :, bco:bco + bw],
                            ps[:, j, 0:18 * bw].rearrange("p (x b) -> p x b", b=bw),
                            0.0,
                        )
                r2t = r2pool.tile([M, 18, _BC], u8, tag="r2", name=f"r2t{bi}")
                nc.gpsimd.tensor_scalar(r2t[:], h2t[:], 0.0, None, is_gt)
                nc.sync.dma_start(h2_dr[bi][:], h2t[:])
                nc.sync.dma_start(r2_dr[bi][:], r2t[:])

            conv1_group(0)
            conv1_group(1)
            conv2_block(0)
            conv1_group(2)
            conv2_block(1)
            conv2_block(2)

    nc.compile()
    return nc


def _get_program():
    with _lock:
        if "nc" not in _cache:
            _cache["nc"] = _build_program()
        return _cache["nc"]


# ---------------------------------------------------------------- entry point

def _maybe_profile_ctx():
    """Optional NTFF capture for local perf iteration (BREW_PROFILE_DIR=...).

    Inert unless the env var is set; the graded path never enters this."""
    outdir = os.environ.get("BREW_PROFILE_DIR")
    if not outdir:
        return contextlib.nullcontext()
    try:
        from trn_agent_boot.trn_boot import _ntff_profile_via_ctypes
        hook = _ntff_profile_via_ctypes("/opt/axon/libaxon_pjrt.so")
        if hook is None:
            return contextlib.nullcontext()
        return hook(outdir, None)
    except Exception:
        return contextlib.nullcontext()


def kernel(x, W1, W2):
    from concourse.bass_utils import run_bass_kernel_spmd

    nc = _get_program()
    in_maps = _host_inputs(np.asarray(x), np.asarray(W1), np.asarray(W2))
    with _maybe_profile_ctx():
        res = run_bass_kernel_spmd(nc, in_maps, core_ids=list(range(_NCORES)))
    return _assemble(res.results)


# revision 12
# speedup vs baseline: 4.8776x; 4.8776x over previous
"""Trainium2 Bass kernel for nn_BrewCnnLayer (2-layer CNN + relu-ratio).

Math (per image, fp32):
  h1b = conv7x7_valid(x.reshape(28,28), W1)   -> (10, 22, 22)
  h1, rat1 = relu(h1b), step(h1b)
  h2b = conv5x5_valid(h1, W2)                 -> (10, 18, 18)
  h2, rat2 = relu(h2b), step(h2b)
returns (h2, rat1, rat2) for the full batch of 2048.

Mapping: pure data parallel over batch across 8 NeuronCores (256 images each).
On-chip, convs are matmuls on the PE array:
  conv1: K = (kx, y_in) Toeplitz patch partitions (<=126), M = (y_out, o) <= 120,
         N = (x, batch) streamed. Input is host-expanded into patch tensors (XE).
  conv2: K = (y_in, c) = h1 rows on partitions, M = (y_out, o), 5 accumulating
         matmuls over kx with shifted x-slices of the h1 tile.
conv1 output rows are grouped exactly as conv2 consumes them (rows 0-11 / 8-19 /
16-21) so the relu epilogue writes land directly in conv2's rhs layout.
rat1/rat2 are computed as uint8 on-chip (step function) to cut DMA volume.
"""

import contextlib
import os
import threading

import numpy as np

_NCORES = 8
_BFULL = 2048
_BC = _BFULL // _NCORES  # 256 images per core

# conv1 groups: (first h1 row, input image rows consumed, h1 rows produced)
_GROUPS = [(0, 18, 12), (8, 18, 12), (16, 12, 6)]
# conv2 blocks: (first h2 row, h1 rows consumed, h2 rows produced)
_BLOCKS = [(0, 12, 8), (8, 12, 8), (16, 6, 2)]
# rat1 slices: (group idx, first row within group, n rows) -> covers h1 rows 0..21 once
_R1CUTS = [(0, 0, 12), (1, 4, 8), (2, 4, 2)]

_lock = threading.Lock()
_cache = {}


# ---------------------------------------------------------------- host prep

def _fp32r_round(a):
    """Round fp32 values to the PE's fp32r format (11-bit mantissa,
    round-to-nearest-even) — bit-identical to neuronxcc's fp32_to_fp32r."""
    u = np.ascontiguousarray(a, np.float32).view(np.uint32)
    low12 = u & np.uint32(0xFFF)
    rnd = (low12 > 0x800) | ((low12 == 0x800)
                             & (((u >> np.uint32(12)) & np.uint32(1)) == np.uint32(1)))
    out = (u & np.uint32(0xFFFFF000)) + np.where(rnd, np.uint32(0x1000),
                                                 np.uint32(0)).astype(np.uint32)
    return out.view(np.float32)


def _build_lhsT1(W1r, Hg, Rg):
    """[7*Hg, Rg*10]: lhsT[(kx,yi),(yr,o)] = W1[o, yi-yr, kx]."""
    L = np.zeros((7, Hg, Rg, 10), np.float32)
    yi = np.arange(Hg)[:, None]
    yr = np.arange(Rg)[None, :]
    ky = yi - yr  # [Hg, Rg]
    valid = (ky >= 0) & (ky < 7)
    for kx in range(7):
        # W1r[:, ky, kx] -> [10] per (yi, yr)
        src = W1r[:, np.clip(ky, 0, 6), kx]          # [10, Hg, Rg]
        L[kx] = np.where(valid[:, :, None], src.transpose(1, 2, 0), 0.0)
    return L.reshape(7 * Hg, Rg * 10)


def _build_lhsT2(W2r, Hb, Ob):
    """[Hb*10, 5, Ob*10]: lhsT[(yi,c), kx, (yr,o)] = W2[o, c, yi-yr, kx]."""
    L = np.zeros((Hb, 10, 5, Ob, 10), np.float32)
    for yi in range(Hb):
        for yr in range(Ob):
            ky = yi - yr
            if 0 <= ky < 5:
                # W2r[o, c, ky, kx] -> [c, kx, o]
                L[yi, :, :, yr, :] = W2r[:, :, ky, :].transpose(1, 2, 0)
    return L.reshape(Hb * 10, 5, Ob * 10)


def _host_inputs(x, W1, W2):
    """Build per-core in_maps for run_bass_kernel_spmd."""
    x = np.ascontiguousarray(x, np.float32)
    W1r = np.asarray(W1, np.float32).reshape(10, 7, 7)
    W2r = np.asarray(W2, np.float32).reshape(10, 10, 5, 5)

    w1ab = _fp32r_round(_build_lhsT1(W1r, 18, 12))   # [126, 120]
    w1c = _fp32r_round(_build_lhsT1(W1r, 12, 6))     # [84, 60]
    w2ab = _fp32r_round(_build_lhsT2(W2r, 12, 8))    # [120, 5, 80]
    w2c = _fp32r_round(_build_lhsT2(W2r, 6, 2))      # [60, 5, 20]

    x2 = _fp32r_round(x).reshape(_BFULL, 28, 28)
    in_maps = []
    for c in range(_NCORES):
        xc = x2[c * _BC:(c + 1) * _BC]          # [BC, 28, 28]
        m = {"w1ab": w1ab, "w1c": w1c, "w2ab": w2ab, "w2c": w2c}
        for gi, (g0, Hg, _Rg) in enumerate(_GROUPS):
            xe = np.empty((7, Hg, 22, _BC), np.float32)
            for kx in range(7):
                xe[kx] = xc[:, g0:g0 + Hg, kx:kx + 22].transpose(1, 2, 0)
            m[f"xe{gi}"] = xe.reshape(7 * Hg, 22, _BC)
        in_maps.append(m)
    return in_maps


def _assemble(results):
    """Per-core result dicts -> (h2, rat1, rat2) full arrays."""
    h2 = np.empty((_BFULL, 10, 18, 18), np.float32)
    rat2 = np.empty((_BFULL, 10, 18, 18), np.float32)
    rat1 = np.empty((_BFULL, 10, 22, 22), np.float32)
    for c, r in enumerate(results):
        bs = slice(c * _BC, (c + 1) * _BC)
        for bi, (b0, _Hb, Ob) in enumerate(_BLOCKS):
            h2s = r[f"h2_{bi}"]                  # [(yr,o), 18, BC]
            r2s = r[f"r2_{bi}"]
            h2[bs, :, b0:b0 + Ob, :] = h2s.reshape(Ob, 10, 18, _BC).transpose(3, 1, 0, 2)
            rat2[bs, :, b0:b0 + Ob, :] = r2s.reshape(Ob, 10, 18, _BC).transpose(3, 1, 0, 2)
        for gi, yr0, nr in _R1CUTS:
            g0 = _GROUPS[gi][0]
            sl = r[f"r1_{gi}"]                   # [nr*10, 22, BC]
            rat1[bs, :, g0 + yr0:g0 + yr0 + nr, :] = sl.reshape(nr, 10, 22, _BC).transpose(3, 1, 0, 2)
    return h2, rat1, rat2


# ---------------------------------------------------------------- bass program

def _build_program():
    import concourse.bass as bass
    import concourse.tile as tile
    import concourse.mybir as mybir
    from concourse import bacc

    f32 = mybir.dt.float32
    f32r = mybir.dt.float32r
    u8 = mybir.dt.uint8
    Relu = mybir.ActivationFunctionType.Relu
    Sign = mybir.ActivationFunctionType.Sign
    is_gt = mybir.AluOpType.is_gt

    nc = bacc.Bacc("TRN2", target_bir_lowering=False, debug=False,
                   num_devices=_NCORES)

    # DRAM I/O (matmul operands are pre-rounded fp32r, supplied by the host)
    xe_dr = [nc.dram_tensor(f"xe{gi}", [7 * Hg, 22, _BC], f32r, kind="ExternalInput").ap()
             for gi, (_g0, Hg, _Rg) in enumerate(_GROUPS)]
    w1ab_dr = nc.dram_tensor("w1ab", [126, 120], f32r, kind="ExternalInput").ap()
    w1c_dr = nc.dram_tensor("w1c", [84, 60], f32r, kind="ExternalInput").ap()
    w2ab_dr = nc.dram_tensor("w2ab", [120, 5, 80], f32r, kind="ExternalInput").ap()
    w2c_dr = nc.dram_tensor("w2c", [60, 5, 20], f32r, kind="ExternalInput").ap()

    h2_dr = [nc.dram_tensor(f"h2_{bi}", [Ob * 10, 18, _BC], f32, kind="ExternalOutput").ap()
             for bi, (_b0, _Hb, Ob) in enumerate(_BLOCKS)]
    r2_dr = [nc.dram_tensor(f"r2_{bi}", [Ob * 10, 18, _BC], u8, kind="ExternalOutput").ap()
             for bi, (_b0, _Hb, Ob) in enumerate(_BLOCKS)]
    r1_dr = [nc.dram_tensor(f"r1_{gi}", [nr * 10, 22, _BC], u8, kind="ExternalOutput").ap()
             for gi, _yr0, nr in _R1CUTS]

    with tile.TileContext(nc) as tc:
        with (
            tc.tile_pool(name="w", bufs=1) as wpool,
            tc.tile_pool(name="xe", bufs=2) as xepool,
            tc.tile_pool(name="h1", bufs=2) as h1pool,
            tc.tile_pool(name="r1", bufs=2) as r1pool,
            tc.tile_pool(name="h2", bufs=2) as h2pool,
            tc.tile_pool(name="r2", bufs=2) as r2pool,
            tc.tile_pool(name="ps1", bufs=2, space="PSUM") as ps1pool,
            tc.tile_pool(name="ps2", bufs=2, space="PSUM") as ps2pool,
        ):
            w1ab_sb = wpool.tile([126, 120], f32r)
            nc.sync.dma_start(w1ab_sb[:], w1ab_dr[:])
            w1c_sb = wpool.tile([84, 60], f32r)
            nc.sync.dma_start(w1c_sb[:], w1c_dr[:])
            w2ab_sb = wpool.tile([120, 5, 80], f32r)
            nc.sync.dma_start(w2ab_sb[:], w2ab_dr[:])
            w2c_sb = wpool.tile([60, 5, 20], f32r)
            nc.sync.dma_start(w2c_sb[:], w2c_dr[:])

            h1_tiles = [None, None, None]

            def conv1_group(gi):
                g0, Hg, Rg = _GROUPS[gi]
                K, M = 7 * Hg, Rg * 10
                w_sb = w1ab_sb if gi < 2 else w1c_sb
                xe_sb = xepool.tile([K, 22, _BC], f32r, tag="xe", name=f"xe_sb{gi}")
                for c0, c1 in ((0, 6), (6, 12), (12, 17), (17, 22)):
                    nc.sync.dma_start(xe_sb[:, c0:c1, :], xe_dr[gi][:, c0:c1, :])
                # h1 is fp32r: the relu epilogue rounds on write, so conv2 can
                # consume the tile directly as a matmul operand.
                h1t = h1pool.tile([M, 22, _BC], f32r, tag="h1", name=f"h1t{gi}")
                for s0 in range(0, 22, 4):
                    w = min(4, 22 - s0)
                    ps = ps1pool.tile([M, 4, _BC], f32, tag="ps1", name=f"ps1_{gi}_{s0}")
                    for j in range(0, w, 2):
                        nc.tensor.matmul(
                            ps[:, j:j + 2, :],
                            w_sb[:],
                            xe_sb[:, s0 + j:s0 + j + 2, :],
                            start=True, stop=True,
                        )
                    nc.scalar.activation(h1t[:, s0:s0 + w, :], ps[:, 0:w, :], Relu)
                h1_tiles[gi] = h1t
                r1t = r1pool.tile([M, 22, _BC], u8, tag="r1", name=f"r1t{gi}")
                nc.vector.tensor_scalar(r1t[:], h1t[:].bitcast(f32), 0.0, None, is_gt)
                _gi, yr0, nr = _R1CUTS[gi]
                nc.sync.dma_start(r1_dr[gi][:], r1t[yr0 * 10:(yr0 + nr) * 10, :, :])

            def conv2_block(bi):
                b0, Hb, Ob = _BLOCKS[bi]
                K, M = Hb * 10, Ob * 10
                w_sb = w2ab_sb if bi < 2 else w2c_sb
                h1t = h1_tiles[bi]
                h2t = h2pool.tile([M, 18, _BC], f32, tag="h2", name=f"h2t{bi}")
                chunks = [(i * 28, 28) for i in range(9)] + [(252, 4)]
                for s in range(0, 10, 2):
                    ps = ps2pool.tile([M, 2, 512], f32, tag="ps2", name=f"ps2_{bi}_{s}")
                    for j in (0, 1):
                        bco, bw = chunks[s + j]
                        for kx in range(5):
                            nc.tensor.matmul(
                                ps[:, j, 0:18 * bw],
                                w_sb[:, kx, :],
                                h1t[:, kx:kx + 18, bco:bco + bw],
                                start=(kx == 0), stop=(kx == 4),
                            )
                        # relu epilogue (DVE) psum -> h2 sbuf
                        nc.vector.tensor_scalar_max(
                            h2t[:, :, bco:bco + bw],
                            ps[:, j, 0:18 * bw].rearrange("p (x b) -> p x b", b=bw),
                            0.0,
                        )
                r2t = r2pool.tile([M, 18, _BC], u8, tag="r2", name=f"r2t{bi}")
                # h2 >= 0, so step(h2) == sign(h2); runs on ACT to balance DVE
                nc.scalar.activation(r2t[:], h2t[:], Sign)
                nc.sync.dma_start(h2_dr[bi][:], h2t[:])
                nc.sync.dma_start(r2_dr[bi][:], r2t[:])

            conv1_group(0)
            conv1_group(1)
            conv2_block(0)
            conv1_group(2)
            conv2_block(1)
            conv2_block(2)

    nc.compile()
    return nc


def _get_program():
    with _lock:
        if "nc" not in _cache:
            _cache["nc"] = _build_program()
        return _cache["nc"]


# ---------------------------------------------------------------- entry point

def _maybe_profile_ctx():
    """Optional NTFF capture for local perf iteration (BREW_PROFILE_DIR=...).

    Inert unless the env var is set; the graded path never enters this."""
    outdir = os.environ.get("BREW_PROFILE_DIR")
    if not outdir:
        return contextlib.nullcontext()
    try:
        from trn_agent_boot.trn_boot import _ntff_profile_via_ctypes
        hook = _ntff_profile_via_ctypes("/opt/axon/libaxon_pjrt.so")
        if hook is None:
            return contextlib.nullcontext()
        return hook(outdir, None)
    except Exception:
        return contextlib.nullcontext()


def kernel(x, W1, W2):
    from concourse.bass_utils import run_bass_kernel_spmd

    nc = _get_program()
    in_maps = _host_inputs(np.asarray(x), np.asarray(W1), np.asarray(W2))
    with _maybe_profile_ctx():
        res = run_bass_kernel_spmd(nc, in_maps, core_ids=list(range(_NCORES)))
    return _assemble(res.results)
